# revision 7
# baseline (speedup 1.0000x reference)
"""Trainium2 Bass kernel for the butterfly-CNN problem (nn_CNNLayer_30296699306356).

Network (see problem reference): input conv (k=2,s=2, 1->8 ch) + 10 butterfly
conv levels (k=2,s=2, channels double each level, relu, zero biases) + a
per-block dense matmul (1024 blocks of [8,2]) at the end.

Strategy (memory-regime; weights are ~358 MB fp32 dominated by levels 8-10):
  - Levels 5..9 run in bf16 (weights + activations, fp32 PSUM accumulation).
    Level 10 weights are float8e3 (e3m4) with per-output-channel scales that
    are folded into fea_dense on the host (relu commutes with positive
    scales), halving the dominant weight stream. Measured rel err ~1.4e-2
    (gate 2e-2, deterministic inputs).
  - Levels in..8 are replicated on all 8 cores; levels 9/10 shard the OUTPUT
    channels (1/8 of the dominant weight traffic per core). w9/w10 are fully
    RESIDENT in SBUF so the weight stream runs as one continuous FIFO.
  - x9 reassembly uses a hand-rolled one-shot all-to-all via
    remote_dma_broadcast (SBUF -> peer SBUF, ~5 us) instead of the gpsimd
    AllGather collective (~50 us of barrier+mesh latency). SPMD slot layout
    is XOR-based: slot j on core r holds core (r XOR j)'s x9 shard, which
    keeps every AP core-id-independent; the host permutes each core's w10
    chunk order to match. D2D engines deliver to tpb (requested ^ 2), so
    cross-die dests are requested pre-swapped (validated by probe.py).
  - Level 10 runs "orientation B" (x9 stationary, fp8 weights moving) with
    4-way PE column tiling: four independent 256-col matmul streams at array
    column offsets 0/32/64/96 accumulate into disjoint PSUM partition groups.
  - Final block einsum on the Vector engine across the 4 partition groups.

kernel(**inputs) takes the FULL unsharded inputs and returns the FULL output.
"""

import ml_dtypes
import numpy as np

NCORES = 8
B = 16
P = 128
C = 8
NLVL = 10
BF16 = ml_dtypes.bfloat16
FP8 = ml_dtypes.float8_e3m4
FP8_SCALE_TARGET = 14.0

_CACHE = {}


# ---------------------------------------------------------------- host prep

def _host_prep(inputs):
    """Build the per-core input maps (numpy only)."""
    ind = np.ascontiguousarray(np.asarray(inputs["in_data"], np.float32))
    f = {l: np.asarray(inputs[f"f{l}"], np.float32) for l in range(1, NLVL + 1)}
    f0 = np.asarray(inputs["in_filter"], np.float32)     # [2, 1, 8]
    fd = np.asarray(inputs["fea_dense"], np.float32)     # [1024, 8, 2]

    shared = {}
    # r0 [32, 64, 16]: r0[row, wHi, b] = in[b, wHi*32 + row]
    shared["r0"] = np.ascontiguousarray(
        ind[:, :, 0].reshape(B, 64, 32).transpose(2, 1, 0))

    # w0 [32, 128]: rows (2*wsub + k), cols (wsub*8 + co)
    w0 = np.zeros((32, 128), np.float32)
    for wsub in range(16):
        for k in range(2):
            w0[2 * wsub + k, wsub * 8:wsub * 8 + 8] = f0[k, 0, :]
    shared["w0"] = w0

    # packed levels 1..4 stacked: wpk [4, 128, 128]
    wpk = np.zeros((4, 128, 128), np.float32)
    for lvl in range(1, 5):
        cin = 2 ** (lvl - 1) * C
        cout = 2 ** lvl * C
        s_out = (128 // cin) // 2
        for wso in range(s_out):
            for k in range(2):
                wsi = 2 * wso + k
                wpk[lvl - 1, wsi * cin:(wsi + 1) * cin,
                    wso * cout:(wso + 1) * cout] = f[lvl][k]
    shared["wpk"] = wpk

    # w5/w6/w7 mega-packed [128, 10752] bf16 (kt-major per level), one tile
    w5v = f[5].astype(BF16).reshape(2, 1, 128, 256)
    w6v = f[6].astype(BF16).reshape(2, 2, 128, 512)
    w7v = f[7].astype(BF16).reshape(2, 4, 128, 1024)
    shared["wmid"] = np.ascontiguousarray(np.concatenate([
        w5v.transpose(2, 0, 1, 3).reshape(128, 512),
        w6v.transpose(2, 0, 1, 3).reshape(128, 2048),
        w7v.transpose(2, 0, 1, 3).reshape(128, 8192)], axis=1))

    # f8 is REPLICATED: co-major chunks [4, 128, kt=16, co=512], kt = k*8 + cit
    f8b = f[8].astype(BF16)
    w8full = np.stack([
        np.ascontiguousarray(
            f8b[:, :, c * 512:(c + 1) * 512]
            .reshape(2, 8, 128, 512).transpose(2, 0, 1, 3).reshape(128, 16, 512))
        for c in range(4)])
    shared["w8"] = w8full

    # f9 output-channel shards, packed into 4-ci-tile chunks:
    # [8, 128, 4, 512]; chunk m = k*4 + q, cit = q*4+j
    w9s = []
    f9b = f[9].astype(BF16)
    for r in range(NCORES):
        blk = f9b[:, :, r * 512:(r + 1) * 512]
        v = blk.reshape(2, 4, 4, 128, 512).transpose(0, 1, 3, 2, 4)
        w9s.append(np.ascontiguousarray(v.reshape(8, 128, 4, 512)))

    # f10 output-channel shards in float8_e3m4 with per-output-channel scales
    # (folded into fea_dense below): [16, 128, 4, 1024] fp8.
    # Chunk m = k*8 + j where j is the XOR exchange SLOT: the input-channel
    # block is q = r ^ j (slot j of the gathered x9 holds core (r^j)'s shard).
    s10 = np.max(np.abs(f[10]), axis=(0, 1)) / FP8_SCALE_TARGET  # [8192]
    f10q = (f[10] / s10[None, None, :]).astype(FP8)
    w10s = []
    for r in range(NCORES):
        v = f10q[:, :, r * 1024:(r + 1) * 1024].reshape(2, 8, 4, 128, 1024)
        chunks = []
        for m in range(16):
            k, j = divmod(m, 8)
            q = r ^ j
            chunks.append(v[k, q].transpose(1, 0, 2))     # [128, 4, 1024]
        w10s.append(np.ascontiguousarray(np.stack(chunks)))

    # fea_dense shard with the fp8 scales folded in, packed for the 4 PE
    # column groups: fdt[32*g + b, o, c] = fd_flat[o, g*256 + c] * s10[...]
    fds = []
    for r in range(NCORES):
        blk = fd[r * 128:(r + 1) * 128]                    # [128, 8, 2]
        flat = blk.transpose(2, 0, 1).reshape(2, 1024)     # [o, 1024]
        flat = flat * s10[r * 1024:(r + 1) * 1024][None, :]
        ft = np.zeros((128, 2, 256), np.float32)
        for g in range(4):
            ft[32 * g:32 * g + B] = np.broadcast_to(
                flat[None, :, 256 * g:256 * (g + 1)], (B, 2, 256))
        fds.append(np.ascontiguousarray(ft))

    in_maps = []
    for r in range(NCORES):
        m = dict(shared)
        m["w9"] = w9s[r]
        m["w10"] = w10s[r]
        m["fdt"] = fds[r]
        in_maps.append(m)
    return in_maps


# ---------------------------------------------------------------- bass build

def _build():
    import concourse.bass as bass
    import concourse.mybir as mybir
    import concourse.tile as tile
    from concourse import bacc

    f32 = mybir.dt.float32
    bf16 = mybir.dt.bfloat16
    fp8 = mybir.dt.float8e3
    RELU = mybir.ActivationFunctionType.Relu

    nc = bacc.Bacc("TRN2", target_bir_lowering=False, debug=False,
                   num_devices=NCORES)

    def inp(name, shape, dt=f32):
        return nc.dram_tensor(name, shape, dt, kind="ExternalInput").ap()

    r0 = inp("r0", [32, 64, 16])
    w0 = inp("w0", [32, 128])
    wpk = inp("wpk", [4, 128, 128])
    wmid = inp("wmid", [128, 10752], bf16)
    w8 = inp("w8", [4, 128, 16, 512], bf16)
    w9 = inp("w9", [8, 128, 4, 512], bf16)
    w10 = inp("w10", [16, 128, 4, 1024], fp8)
    fdt = inp("fdt", [128, 2, 256])
    out = nc.dram_tensor("out", [B, 128, 2], f32, kind="ExternalOutput").ap()

    xsem = nc.alloc_semaphore("x9_xsem")
    lsem = nc.alloc_semaphore("x9_lsem")
    psem = nc.alloc_semaphore("x9_psem")

    with tile.TileContext(nc) as tc:
        with (
            tc.tile_pool(name="const", bufs=1) as constp,
            tc.tile_pool(name="actp", bufs=3) as actp,
            tc.tile_pool(name="bigp", bufs=1) as bigp,
            tc.tile_pool(name="w7p", bufs=1) as w7p,
            tc.tile_pool(name="w8p", bufs=3) as w8p,
            tc.tile_pool(name="w9p", bufs=1) as w9p,
            tc.tile_pool(name="w10p", bufs=1) as w10p,
            tc.tile_pool(name="psA", bufs=2, space="PSUM") as psA,
            tc.tile_pool(name="psB", bufs=4, space="PSUM") as psB,
            tc.tile_pool(name="psC", bufs=1, space="PSUM") as psC,
        ):
            # ---- resident loads, issued in consumption order
            r0sb = constp.tile([32, 64, 16], f32, name="r0sb")
            nc.sync.dma_start(r0sb[:], r0)
            w0sb = constp.tile([32, 128], f32, name="w0sb")
            nc.sync.dma_start(w0sb[:], w0)
            wpksb = constp.tile([128, 4, 128], f32, name="wpksb")
            nc.sync.dma_start(wpksb[:], wpk.rearrange("l p c -> p l c"))
            wmidsb = w7p.tile([128, 10752], bf16, name="wmidsb")
            # split so l5 can start before w6/w7 land
            nc.sync.dma_start(wmidsb[:, 0:512], wmid[:, 0:512])
            nc.sync.dma_start(wmidsb[:, 512:2560], wmid[:, 512:2560])
            nc.sync.dma_start(wmidsb[:, 2560:6656], wmid[:, 2560:6656])
            nc.sync.dma_start(wmidsb[:, 6656:10752], wmid[:, 6656:10752])
            w5sb = wmidsb[:, 0:512].rearrange("p (t c) -> p t c", c=256)
            w6sb = wmidsb[:, 512:2560].rearrange("p (t c) -> p t c", c=512)
            w7sb = wmidsb[:, 2560:10752].rearrange("p (t c) -> p t c", c=1024)

            # w9/w10 fully resident; slice DMAs let consumers start per-slice
            w9sb = w9p.tile([128, 8, 4, 512], bf16, name="w9sb")
            w10sb = w10p.tile([128, 16, 4, 1024], fp8, name="w10sb")

            # ---- input conv + packed levels 1..4 (all [128, 64, 16])
            xprev = None
            for lvl in range(5):
                # x4 feeds the bf16 level-5 matmul, so cast at the relu
                xn = actp.tile([128, 64, 16], bf16 if lvl == 4 else f32,
                               name=f"x{lvl}", tag="xl")
                for ch in range(2):
                    ps = psA.tile([128, 32, 16], f32, name="psA", tag="psA")
                    if lvl == 0:
                        nc.tensor.matmul(
                            ps[:], w0sb[:], r0sb[:, ch * 32:(ch + 1) * 32, :],
                            start=True, stop=True)
                    else:
                        nc.tensor.matmul(
                            ps[:], wpksb[:, lvl - 1, :],
                            xprev[:, ch * 32:(ch + 1) * 32, :],
                            start=True, stop=True)
                    nc.scalar.activation(
                        xn[:, ch * 32:(ch + 1) * 32, :], ps[:], RELU)
                xprev = xn

            # ---- standard levels (orientation A, weights stationary)
            def std_level(xin, wsb, cin_t, cout_t, w_out, name, out_tile=None):
                # xin [128, cin_t, 2*w_out, 16]; wsb [128, 2*cin_t, co] with
                # kt = k*cin_t + cit; returns [128, cout_t, w_out, 16]
                if out_tile is None:
                    xn = actp.tile([128, cout_t, w_out, 16], bf16,
                                   name=name, tag="xl")
                else:
                    xn = out_tile
                for ct in range(cout_t):
                    ps = psA.tile([128, w_out, 16], f32, name="psA", tag="psA")
                    for cit in range(cin_t):
                        rhs2 = xin[:, cit].rearrange(
                            "p (w two) b -> p two w b", two=2)
                        for k in range(2):
                            nc.tensor.matmul(
                                ps[:],
                                wsb[:, k * cin_t + cit,
                                    ct * 128:(ct + 1) * 128],
                                rhs2[:, k],
                                start=(cit == 0 and k == 0),
                                stop=(cit == cin_t - 1 and k == 1))
                    nc.scalar.activation(xn[:, ct], ps[:], RELU)
                return xn

            x5 = std_level(xprev[:, None], w5sb, 1, 2, 32, "x5")
            x6 = std_level(x5, w6sb, 2, 4, 16, "x6")
            x7 = std_level(x6, w7sb, 4, 8, 8, "x7")

            # ---- level 8 REPLICATED (full 2048 cout), co-major weight stream
            x8sb = bigp.tile([128, 16, 4, 16], bf16, name="x8sb")
            w8cs = []
            for c in range(4):
                w8c = w8p.tile([128, 16, 512], bf16, name="w8c", tag="w8c")
                nc.sync.dma_start(w8c[:], w8[c])
                w8cs.append(w8c)
            # w9/w10/fdt descriptors queue behind w8 on the sync engine
            for m in range(8):
                nc.sync.dma_start(w9sb[:, m], w9[m])
            for m in range(16):
                nc.sync.dma_start(w10sb[:, m], w10[m])
            fdsb = constp.tile([128, 2, 256], f32, name="fdsb")
            nc.sync.dma_start(fdsb[:], fdt)

            for c in range(4):
                w8c = w8cs[c]
                for ctl in range(4):
                    ps = psA.tile([128, 4, 16], f32, name="psA", tag="psA")
                    for cit in range(8):
                        rhs2 = x7[:, cit].rearrange(
                            "p (w two) b -> p two w b", two=2)
                        for k in range(2):
                            nc.tensor.matmul(
                                ps[:],
                                w8c[:, k * 8 + cit, ctl * 128:(ctl + 1) * 128],
                                rhs2[:, k],
                                start=(cit == 0 and k == 0),
                                stop=(cit == 7 and k == 1))
                    nc.scalar.activation(x8sb[:, c * 4 + ctl], ps[:], RELU)

            # ---- level 9 (512-ch shard, resident weights, 4 accumulators)
            ps9 = [psB.tile([128, 2, 16], f32, name=f"ps9_{ct}", tag="psB")
                   for ct in range(4)]
            for m in range(8):
                k, q = divmod(m, 4)
                for j in range(4):
                    cit = q * 4 + j
                    rhs = x8sb[:, cit].rearrange(
                        "p (w two) b -> p two w b", two=2)[:, k]
                    for ct in range(4):
                        nc.tensor.matmul(
                            ps9[ct][:],
                            w9sb[:, m, j, ct * 128:(ct + 1) * 128],
                            rhs,
                            start=(m == 0 and j == 0),
                            stop=(m == 7 and j == 3))

            # ---- x9 all-to-all exchange (XOR slots): x9x[:, j] holds core
            # (r^j)'s [128, 4, 2, 16] shard; slot 0 is written locally.
            x9x = bigp.tile([128, 8, 4, 2, 16], bf16, name="x9x")
            for ct in range(4):
                nc.scalar.activation(x9x[:, 0, ct], ps9[ct][:], RELU)

            # No inter-core entry barrier needed: invocations are
            # host-serialized, xsem is cleared only post-consumption, and
            # early increments accumulate harmlessly. Descriptor generation
            # (slow, ~6us + ucode lib load) runs EARLY in its own critical;
            # Tile defers the source-tensor dep to the trigger, which fires
            # as soon as x9x slot 0 is written. no_gpsimd_drain skips the
            # ~44us SWDGE quiesce at critical exit (sends' completion is
            # proven via lsem instead).
            with tc.tile_critical(no_gpsimd_drain=True):
                nc.gpsimd.sem_clear(psem)
                nc.gpsimd.sem_clear(lsem)
                for i in range(1, NCORES):
                    rd = [None] * 8
                    # D2D engines deliver to tpb (requested ^ 2): pre-swap
                    v = i ^ 2 if i & 4 else i
                    rd[v] = (0, v)
                    nc.gpsimd.remote_dma_broadcast(
                        x9x[:, i], x9x[:, 0],
                        remote_sem=xsem, local_sem=lsem, rdests=rd
                    ).then_inc(psem, 1)

            with tc.tile_critical(no_gpsimd_drain=True):
                nc.gpsimd.wait_ge(psem, 7)
                nc.gpsimd.trigger_dma(count=7)
                nc.gpsimd.wait_ge(lsem, 112)

            x9sb = bigp.tile([128, 8, 4, 2, 16], bf16, name="x9sb")
            with tc.tile_critical(no_gpsimd_drain=True):
                nc.vector.wait_ge(xsem, 14)
                nc.vector.sem_clear(xsem)
                nc.vector.tensor_scalar_add(x9sb[:], x9x[:], 0.0)

            # ---- level 10 (1024-ch shard, orientation B, fp8 weights moving,
            #      4-way PE column tiling: group g -> array cols 32g, PSUM
            #      partitions [32g, 32g+16), output cols [256g, 256(g+1)))
            ps10 = psC.tile([128, 256], f32, name="ps10")
            for m in range(16):
                k, j = divmod(m, 8)
                xsrc = x9x if j == 0 else x9sb
                for jj in range(4):
                    lhsT = xsrc[:, j, jj, k, :]
                    for g in range(4):
                        nc.tensor.matmul(
                            ps10[32 * g:32 * g + B, :], lhsT,
                            w10sb[:, m, jj, 256 * g:256 * (g + 1)],
                            start=(m == 0 and jj == 0),
                            stop=(m == 15 and jj == 3),
                            tile_position=(0, 32 * g),
                            skip_group_check=True)
            x10 = bigp.tile([128, 256], f32, name="x10")
            for g in range(4):
                nc.scalar.activation(
                    x10[32 * g:32 * g + B, :], ps10[32 * g:32 * g + B, :],
                    RELU)

            # ---- final per-block einsum on the vector engine
            osb = bigp.tile([128, 32, 2], f32, name="osb")
            for o in range(2):
                prod = bigp.tile([128, 256], f32, name=f"prod{o}")
                nc.vector.tensor_tensor(
                    prod[:], x10[:], fdsb[:, o, :], mybir.AluOpType.mult)
                nc.vector.tensor_reduce(
                    osb[:, :, o],
                    prod.rearrange("p (k c) -> p k c", c=8),
                    mybir.AxisListType.X, mybir.AluOpType.add)
            for g in range(4):
                nc.sync.dma_start(out[:, 32 * g:32 * (g + 1), :],
                                  osb[32 * g:32 * g + B, :, :])

    nc.compile()
    return nc


# ------------------------------------------------------------------- kernel

def kernel(**inputs):
    from concourse.bass_utils import run_bass_kernel_spmd

    in_maps = _host_prep(inputs)
    if "nc" not in _CACHE:
        _CACHE["nc"] = _build()
    nc = _CACHE["nc"]
    res = run_bass_kernel_spmd(nc, in_maps, core_ids=list(range(NCORES)))
    parts = [res.results[r]["out"] for r in range(NCORES)]  # each [16, 128, 2]
    full = np.concatenate(parts, axis=1)                    # [16, 1024, 2]
    return np.ascontiguousarray(full.reshape(B, 2048, 1).astype(np.float32))


# revision 13
# speedup vs baseline: 50.0049x; 50.0049x over previous
"""Trainium2 Bass kernel for the butterfly-CNN problem (nn_CNNLayer_30296699306356).

Network (see problem reference): input conv (k=2,s=2, 1->8 ch) + 10 butterfly
conv levels (k=2,s=2, channels double each level, relu, zero biases) + a
per-block dense matmul (1024 blocks of [8,2]) at the end.

Strategy (memory-regime; weights are ~358 MB fp32 dominated by levels 8-10):
  - Levels 5..9 run in bf16 (weights + activations, fp32 PSUM accumulation).
    Level 10 weights are float8e3 (e3m4) with per-output-channel scales that
    are folded into fea_dense on the host (relu commutes with positive
    scales), halving the dominant weight stream. Measured rel err ~1.4e-2
    (gate 2e-2, deterministic inputs).
  - Levels in..8 are replicated on all 8 cores; levels 9/10 shard the OUTPUT
    channels (1/8 of the dominant weight traffic per core). w9/w10 are fully
    RESIDENT in SBUF so the weight stream runs as one continuous FIFO.
  - x9 reassembly uses a hand-rolled one-shot all-to-all via
    remote_dma_broadcast (SBUF -> peer SBUF, ~5 us) instead of the gpsimd
    AllGather collective (~50 us of barrier+mesh latency). SPMD slot layout
    is XOR-based: slot j on core r holds core (r XOR j)'s x9 shard, which
    keeps every AP core-id-independent; the host permutes each core's w10
    chunk order to match. D2D engines deliver to tpb (requested ^ 2), so
    cross-die dests are requested pre-swapped (validated by probe.py).
  - Level 10 runs "orientation B" (x9 stationary, fp8 weights moving) with
    4-way PE column tiling: four independent 256-col matmul streams at array
    column offsets 0/32/64/96 accumulate into disjoint PSUM partition groups.
  - Final block einsum on the Vector engine across the 4 partition groups.

kernel(**inputs) takes the FULL unsharded inputs and returns the FULL output.
"""

import ml_dtypes
import numpy as np

NCORES = 8
B = 16
P = 128
C = 8
NLVL = 10
BF16 = ml_dtypes.bfloat16
FP8 = ml_dtypes.float8_e3m4
FP8_SCALE_TARGET = 14.0

_CACHE = {}


# ---------------------------------------------------------------- host prep

def _host_prep(inputs):
    """Build the per-core input maps (numpy only)."""
    ind = np.ascontiguousarray(np.asarray(inputs["in_data"], np.float32))
    f = {l: np.asarray(inputs[f"f{l}"], np.float32) for l in range(1, NLVL + 1)}
    f0 = np.asarray(inputs["in_filter"], np.float32)     # [2, 1, 8]
    fd = np.asarray(inputs["fea_dense"], np.float32)     # [1024, 8, 2]

    shared = {}
    # r0 [32, 64, 16]: r0[row, wHi, b] = in[b, wHi*32 + row]
    shared["r0"] = np.ascontiguousarray(
        ind[:, :, 0].reshape(B, 64, 32).transpose(2, 1, 0))

    # w0 [32, 128]: rows (2*wsub + k), cols (wsub*8 + co)
    w0 = np.zeros((32, 128), np.float32)
    for wsub in range(16):
        for k in range(2):
            w0[2 * wsub + k, wsub * 8:wsub * 8 + 8] = f0[k, 0, :]
    shared["w0"] = w0

    # packed levels 1..4 stacked: wpk [4, 128, 128]
    wpk = np.zeros((4, 128, 128), np.float32)
    for lvl in range(1, 5):
        cin = 2 ** (lvl - 1) * C
        cout = 2 ** lvl * C
        s_out = (128 // cin) // 2
        for wso in range(s_out):
            for k in range(2):
                wsi = 2 * wso + k
                wpk[lvl - 1, wsi * cin:(wsi + 1) * cin,
                    wso * cout:(wso + 1) * cout] = f[lvl][k]
    shared["wpk"] = wpk

    # w5/w6/w7 mega-packed [128, 10752] bf16 (kt-major per level), one tile
    w5v = f[5].astype(BF16).reshape(2, 1, 128, 256)
    w6v = f[6].astype(BF16).reshape(2, 2, 128, 512)
    w7v = f[7].astype(BF16).reshape(2, 4, 128, 1024)
    shared["wmid"] = np.ascontiguousarray(np.concatenate([
        w5v.transpose(2, 0, 1, 3).reshape(128, 512),
        w6v.transpose(2, 0, 1, 3).reshape(128, 2048),
        w7v.transpose(2, 0, 1, 3).reshape(128, 8192)], axis=1))

    # f8 is REPLICATED: co-major chunks [4, 128, kt=16, co=512], kt = k*8 + cit
    f8b = f[8].astype(BF16)
    w8full = np.stack([
        np.ascontiguousarray(
            f8b[:, :, c * 512:(c + 1) * 512]
            .reshape(2, 8, 128, 512).transpose(2, 0, 1, 3).reshape(128, 16, 512))
        for c in range(4)])
    shared["w8"] = w8full

    # f9 output-channel shards, packed into 4-ci-tile chunks:
    # [8, 128, 4, 512]; chunk m = k*4 + q, cit = q*4+j
    w9s = []
    f9b = f[9].astype(BF16)
    for r in range(NCORES):
        blk = f9b[:, :, r * 512:(r + 1) * 512]
        v = blk.reshape(2, 4, 4, 128, 512).transpose(0, 1, 3, 2, 4)
        w9s.append(np.ascontiguousarray(v.reshape(8, 128, 4, 512)))

    # f10 output-channel shards in float8_e3m4 with per-output-channel scales
    # (folded into fea_dense below): [16, 128, 4, 1024] fp8.
    # Chunk m = k*8 + j where j is the XOR exchange SLOT: the input-channel
    # block is q = r ^ j (slot j of the gathered x9 holds core (r^j)'s shard).
    s10 = np.max(np.abs(f[10]), axis=(0, 1)) / FP8_SCALE_TARGET  # [8192]
    f10q = (f[10] / s10[None, None, :]).astype(FP8)
    w10s = []
    for r in range(NCORES):
        v = f10q[:, :, r * 1024:(r + 1) * 1024].reshape(2, 8, 4, 128, 1024)
        chunks = []
        for m in range(16):
            k, j = divmod(m, 8)
            q = r ^ j
            chunks.append(v[k, q].transpose(1, 0, 2))     # [128, 4, 1024]
        w10s.append(np.ascontiguousarray(np.stack(chunks)))

    # fea_dense shard with the fp8 scales folded in, packed for the 4 PE
    # column groups: fdt[32*g + b, o, c] = fd_flat[o, g*256 + c] * s10[...]
    fds = []
    for r in range(NCORES):
        blk = fd[r * 128:(r + 1) * 128]                    # [128, 8, 2]
        flat = blk.transpose(2, 0, 1).reshape(2, 1024)     # [o, 1024]
        flat = flat * s10[r * 1024:(r + 1) * 1024][None, :]
        ft = np.zeros((128, 2, 256), np.float32)
        for g in range(4):
            ft[32 * g:32 * g + B] = np.broadcast_to(
                flat[None, :, 256 * g:256 * (g + 1)], (B, 2, 256))
        fds.append(np.ascontiguousarray(ft))

    in_maps = []
    for r in range(NCORES):
        m = dict(shared)
        m["w9"] = w9s[r]
        m["w10"] = w10s[r]
        m["fdt"] = fds[r]
        in_maps.append(m)
    return in_maps


# ---------------------------------------------------------------- bass build

def _build():
    import concourse.bass as bass
    import concourse.mybir as mybir
    import concourse.tile as tile
    from concourse import bacc

    f32 = mybir.dt.float32
    bf16 = mybir.dt.bfloat16
    fp8 = mybir.dt.float8e3
    RELU = mybir.ActivationFunctionType.Relu

    nc = bacc.Bacc("TRN2", target_bir_lowering=False, debug=False,
                   num_devices=NCORES)

    def inp(name, shape, dt=f32):
        return nc.dram_tensor(name, shape, dt, kind="ExternalInput").ap()

    r0 = inp("r0", [32, 64, 16])
    w0 = inp("w0", [32, 128])
    wpk = inp("wpk", [4, 128, 128])
    wmid = inp("wmid", [128, 10752], bf16)
    w8 = inp("w8", [4, 128, 16, 512], bf16)
    w9 = inp("w9", [8, 128, 4, 512], bf16)
    w10 = inp("w10", [16, 128, 4, 1024], fp8)
    fdt = inp("fdt", [128, 2, 256])
    out = nc.dram_tensor("out", [B, 128, 2], f32, kind="ExternalOutput").ap()

    xsem = nc.alloc_semaphore("x9_xsem")
    lsem = nc.alloc_semaphore("x9_lsem")
    psem = nc.alloc_semaphore("x9_psem")

    with tile.TileContext(nc) as tc:
        with (
            tc.tile_pool(name="const", bufs=1) as constp,
            tc.tile_pool(name="actp", bufs=3) as actp,
            tc.tile_pool(name="bigp", bufs=1) as bigp,
            tc.tile_pool(name="w7p", bufs=1) as w7p,
            tc.tile_pool(name="w8p", bufs=3) as w8p,
            tc.tile_pool(name="w9p", bufs=1) as w9p,
            tc.tile_pool(name="w10p", bufs=1) as w10p,
            tc.tile_pool(name="psA", bufs=2, space="PSUM") as psA,
            tc.tile_pool(name="psB", bufs=4, space="PSUM") as psB,
            tc.tile_pool(name="psC", bufs=1, space="PSUM") as psC,
            tc.tile_pool(name="dramp", bufs=1, space="DRAM") as dramp,
        ):
            # 1-byte AllGather at kernel entry, consumed by nobody: its
            # presence makes the runtime gang-dispatch the 8 cores (without
            # any collective, per-core launches stagger by up to
            # milliseconds, which the exchange waits then absorb).
            agb_in = dramp.tile([1, 4], mybir.dt.uint8, name="agb_in")
            agb_out = dramp.tile([NCORES, 4], mybir.dt.uint8, name="agb_out",
                                 addr_space="Shared")
            nc.gpsimd.collective_compute(
                "AllGather", mybir.AluOpType.bypass,
                replica_groups=[list(range(NCORES))],
                ins=[agb_in.opt()], outs=[agb_out.opt()])
            # ---- resident loads, issued in consumption order
            r0sb = constp.tile([32, 64, 16], f32, name="r0sb")
            nc.sync.dma_start(r0sb[:], r0)
            w0sb = constp.tile([32, 128], f32, name="w0sb")
            nc.sync.dma_start(w0sb[:], w0)
            wpksb = constp.tile([128, 4, 128], f32, name="wpksb")
            nc.sync.dma_start(wpksb[:], wpk.rearrange("l p c -> p l c"))
            wmidsb = w7p.tile([128, 10752], bf16, name="wmidsb")
            # split so l5 can start before w6/w7 land
            nc.sync.dma_start(wmidsb[:, 0:512], wmid[:, 0:512])
            nc.sync.dma_start(wmidsb[:, 512:2560], wmid[:, 512:2560])
            nc.sync.dma_start(wmidsb[:, 2560:6656], wmid[:, 2560:6656])
            nc.sync.dma_start(wmidsb[:, 6656:10752], wmid[:, 6656:10752])
            w5sb = wmidsb[:, 0:512].rearrange("p (t c) -> p t c", c=256)
            w6sb = wmidsb[:, 512:2560].rearrange("p (t c) -> p t c", c=512)
            w7sb = wmidsb[:, 2560:10752].rearrange("p (t c) -> p t c", c=1024)

            # w9/w10 fully resident; slice DMAs let consumers start per-slice
            w9sb = w9p.tile([128, 8, 4, 512], bf16, name="w9sb")
            w10sb = w10p.tile([128, 16, 4, 1024], fp8, name="w10sb")

            # x9 exchange buffer (XOR slots): x9x[:, j] holds core (r^j)'s
            # [128, 4, 2, 16] shard; slot 0 is written locally by l9.
            x9x = bigp.tile([128, 8, 4, 2, 16], bf16, name="x9x")

            # Exchange descriptor generation is slow (~6us + gpsimd ucode lib
            # load): run it EARLY in its own critical. Tile defers the
            # source-tensor dep of each prep to the trigger (in a later
            # critical), so this one has no data deps. Criticals are
            # all-engine program-order barriers, hence the early placement.
            # no_gpsimd_drain skips a ~44us SWDGE quiesce at critical exit.
            with tc.tile_critical(no_gpsimd_drain=True):
                nc.gpsimd.sem_clear(psem)
                for i in range(1, NCORES):
                    rd = [None] * 8
                    # D2D engines deliver to tpb (requested ^ 2): pre-swap
                    v = i ^ 2 if i & 4 else i
                    rd[v] = (0, v)
                    nc.gpsimd.remote_dma_broadcast(
                        x9x[:, i], x9x[:, 0],
                        remote_sem=xsem, local_sem=lsem, rdests=rd
                    ).then_inc(psem, 1)

            # ---- input conv + packed levels 1..4 (all [128, 64, 16])
            xprev = None
            for lvl in range(5):
                # x4 feeds the bf16 level-5 matmul, so cast at the relu
                xn = actp.tile([128, 64, 16], bf16 if lvl == 4 else f32,
                               name=f"x{lvl}", tag="xl")
                for ch in range(2):
                    ps = psA.tile([128, 32, 16], f32, name="psA", tag="psA")
                    if lvl == 0:
                        nc.tensor.matmul(
                            ps[:], w0sb[:], r0sb[:, ch * 32:(ch + 1) * 32, :],
                            start=True, stop=True)
                    else:
                        nc.tensor.matmul(
                            ps[:], wpksb[:, lvl - 1, :],
                            xprev[:, ch * 32:(ch + 1) * 32, :],
                            start=True, stop=True)
                    nc.scalar.activation(
                        xn[:, ch * 32:(ch + 1) * 32, :], ps[:], RELU)
                xprev = xn

            # ---- standard levels (orientation A, weights stationary)
            def std_level(xin, wsb, cin_t, cout_t, w_out, name, out_tile=None):
                # xin [128, cin_t, 2*w_out, 16]; wsb [128, 2*cin_t, co] with
                # kt = k*cin_t + cit; returns [128, cout_t, w_out, 16]
                if out_tile is None:
                    xn = actp.tile([128, cout_t, w_out, 16], bf16,
                                   name=name, tag="xl")
                else:
                    xn = out_tile
                for ct in range(cout_t):
                    ps = psA.tile([128, w_out, 16], f32, name="psA", tag="psA")
                    for cit in range(cin_t):
                        rhs2 = xin[:, cit].rearrange(
                            "p (w two) b -> p two w b", two=2)
                        for k in range(2):
                            nc.tensor.matmul(
                                ps[:],
                                wsb[:, k * cin_t + cit,
                                    ct * 128:(ct + 1) * 128],
                                rhs2[:, k],
                                start=(cit == 0 and k == 0),
                                stop=(cit == cin_t - 1 and k == 1))
                    nc.scalar.activation(xn[:, ct], ps[:], RELU)
                return xn

            x5 = std_level(xprev[:, None], w5sb, 1, 2, 32, "x5")
            x6 = std_level(x5, w6sb, 2, 4, 16, "x6")
            x7 = std_level(x6, w7sb, 4, 8, 8, "x7")

            # ---- level 8 REPLICATED (full 2048 cout), co-major weight stream
            x8sb = bigp.tile([128, 16, 4, 16], bf16, name="x8sb")
            w8cs = []
            for c in range(4):
                w8c = w8p.tile([128, 16, 512], bf16, name="w8c", tag="w8c")
                nc.sync.dma_start(w8c[:], w8[c])
                w8cs.append(w8c)
            # w9/w10/fdt descriptors queue behind w8 on the sync engine
            for m in range(8):
                nc.sync.dma_start(w9sb[:, m], w9[m])
            for m in range(16):
                nc.sync.dma_start(w10sb[:, m], w10[m])
            fdsb = constp.tile([128, 2, 256], f32, name="fdsb")
            nc.sync.dma_start(fdsb[:], fdt)

            for c in range(4):
                w8c = w8cs[c]
                for ctl in range(4):
                    ps = psA.tile([128, 4, 16], f32, name="psA", tag="psA")
                    for cit in range(8):
                        rhs2 = x7[:, cit].rearrange(
                            "p (w two) b -> p two w b", two=2)
                        for k in range(2):
                            nc.tensor.matmul(
                                ps[:],
                                w8c[:, k * 8 + cit, ctl * 128:(ctl + 1) * 128],
                                rhs2[:, k],
                                start=(cit == 0 and k == 0),
                                stop=(cit == 7 and k == 1))
                    nc.scalar.activation(x8sb[:, c * 4 + ctl], ps[:], RELU)

            # ---- level 9 (512-ch shard, resident weights, 4 accumulators)
            ps9 = [psB.tile([128, 2, 16], f32, name=f"ps9_{ct}", tag="psB")
                   for ct in range(4)]
            for m in range(8):
                k, q = divmod(m, 4)
                for j in range(4):
                    cit = q * 4 + j
                    rhs = x8sb[:, cit].rearrange(
                        "p (w two) b -> p two w b", two=2)[:, k]
                    for ct in range(4):
                        nc.tensor.matmul(
                            ps9[ct][:],
                            w9sb[:, m, j, ct * 128:(ct + 1) * 128],
                            rhs,
                            start=(m == 0 and j == 0),
                            stop=(m == 7 and j == 3))

            # ---- x9 local shard -> x9x slot 0, then fire the exchange.
            # No inter-core entry barrier needed: invocations are
            # host-serialized, xsem is cleared only post-consumption, and
            # early increments accumulate harmlessly.
            for ct in range(4):
                nc.scalar.activation(x9x[:, 0, ct], ps9[ct][:], RELU)

            # The token copy's read of x9x slot 0 gates the critical's entry
            # (the trigger instruction itself carries no tensor inputs, so
            # without it the sends would fire before l9's output exists).
            x9tok = bigp.tile([128, 4, 2, 16], bf16, name="x9tok")
            with tc.tile_critical(no_gpsimd_drain=True):
                nc.vector.tensor_scalar_add(x9tok[:], x9x[:, 0], 0.0)
                nc.gpsimd.wait_ge(psem, 7)
                nc.gpsimd.trigger_dma(count=7)

            # ---- level 10 (1024-ch shard, orientation B, fp8 weights moving,
            #      4-way PE column tiling: group g -> array cols 32g, PSUM
            #      partitions [32g, 32g+16), output cols [256g, 256(g+1))).
            #      Slot-0 (local) chunks run before the exchange completes.
            ps10 = psC.tile([128, 256], f32, name="ps10")

            def l10_chunk(m, xsrc):
                k, j = divmod(m, 8)
                for jj in range(4):
                    lhsT = xsrc[:, j, jj, k, :]
                    for g in range(4):
                        nc.tensor.matmul(
                            ps10[32 * g:32 * g + B, :], lhsT,
                            w10sb[:, m, jj, 256 * g:256 * (g + 1)],
                            start=(m == 0 and jj == 0),
                            stop=(m == 15 and jj == 3),
                            tile_position=(0, 32 * g),
                            skip_group_check=True)

            l10_chunk(0, x9x)
            l10_chunk(8, x9x)

            x9sb = bigp.tile([128, 8, 4, 2, 16], bf16, name="x9sb")
            with tc.tile_critical(no_gpsimd_drain=True):
                nc.vector.wait_ge(xsem, 14)
                nc.vector.sem_clear(xsem)
                nc.vector.tensor_scalar_add(x9sb[:], x9x[:], 0.0)

            for m in range(16):
                if m not in (0, 8):
                    l10_chunk(m, x9sb)
            x10 = bigp.tile([128, 256], f32, name="x10")
            for g in range(4):
                nc.scalar.activation(
                    x10[32 * g:32 * g + B, :], ps10[32 * g:32 * g + B, :],
                    RELU)

            # ---- final per-block einsum on the vector engine
            osb = bigp.tile([128, 32, 2], f32, name="osb")
            for o in range(2):
                prod = bigp.tile([128, 256], f32, name=f"prod{o}")
                nc.vector.tensor_tensor(
                    prod[:], x10[:], fdsb[:, o, :], mybir.AluOpType.mult)
                nc.vector.tensor_reduce(
                    osb[:, :, o],
                    prod.rearrange("p (k c) -> p k c", c=8),
                    mybir.AxisListType.X, mybir.AluOpType.add)
            for g in range(4):
                nc.sync.dma_start(out[:, 32 * g:32 * (g + 1), :],
                                  osb[32 * g:32 * g + B, :, :])

    nc.compile()
    return nc


# ------------------------------------------------------------------- kernel

def kernel(**inputs):
    from concourse.bass_utils import run_bass_kernel_spmd

    in_maps = _host_prep(inputs)
    if "nc" not in _CACHE:
        _CACHE["nc"] = _build()
    nc = _CACHE["nc"]
    res = run_bass_kernel_spmd(nc, in_maps, core_ids=list(range(NCORES)))
    parts = [res.results[r]["out"] for r in range(NCORES)]  # each [16, 128, 2]
    full = np.concatenate(parts, axis=1)                    # [16, 1024, 2]
    return np.ascontiguousarray(full.reshape(B, 2048, 1).astype(np.float32))


# revision 19
# speedup vs baseline: 50.0779x; 1.0015x over previous
"""Trainium2 Bass kernel for the butterfly-CNN problem (nn_CNNLayer_30296699306356).

Network (see problem reference): input conv (k=2,s=2, 1->8 ch) + 10 butterfly
conv levels (k=2,s=2, channels double each level, relu, zero biases) + a
per-block dense matmul (1024 blocks of [8,2]) at the end.

Strategy (memory-regime; weights are ~358 MB fp32 dominated by levels 8-10):
  - Levels 5..9 run in bf16 (weights + activations, fp32 PSUM accumulation).
    Level 10 weights are float8e3 (e3m4) with per-output-channel scales that
    are folded into fea_dense on the host (relu commutes with positive
    scales), halving the dominant weight stream. Measured rel err ~1.4e-2
    (gate 2e-2, deterministic inputs).
  - Levels in..8 are replicated on all 8 cores; levels 9/10 shard the OUTPUT
    channels (1/8 of the dominant weight traffic per core). w9/w10 are fully
    RESIDENT in SBUF so the weight stream runs as one continuous FIFO.
  - x9 reassembly uses a hand-rolled one-shot all-to-all via
    remote_dma_broadcast (SBUF -> peer SBUF, ~5 us) instead of the gpsimd
    AllGather collective (~50 us of barrier+mesh latency). SPMD slot layout
    is XOR-based: slot j on core r holds core (r XOR j)'s x9 shard, which
    keeps every AP core-id-independent; the host permutes each core's w10
    chunk order to match. D2D engines deliver to tpb (requested ^ 2), so
    cross-die dests are requested pre-swapped (validated by probe.py).
  - Level 10 runs "orientation B" (x9 stationary, fp8 weights moving) with
    4-way PE column tiling: four independent 256-col matmul streams at array
    column offsets 0/32/64/96 accumulate into disjoint PSUM partition groups.
  - Final block einsum on the Vector engine across the 4 partition groups.

kernel(**inputs) takes the FULL unsharded inputs and returns the FULL output.
"""

import ml_dtypes
import numpy as np

NCORES = 8
B = 16
P = 128
C = 8
NLVL = 10
BF16 = ml_dtypes.bfloat16
FP8 = ml_dtypes.float8_e3m4
FP8_SCALE_TARGET = 14.0

_CACHE = {}


# ---------------------------------------------------------------- host prep

def _host_prep(inputs):
    """Build the per-core input maps (numpy only)."""
    ind = np.ascontiguousarray(np.asarray(inputs["in_data"], np.float32))
    f = {l: np.asarray(inputs[f"f{l}"], np.float32) for l in range(1, NLVL + 1)}
    f0 = np.asarray(inputs["in_filter"], np.float32)     # [2, 1, 8]
    fd = np.asarray(inputs["fea_dense"], np.float32)     # [1024, 8, 2]

    shared = {}
    # r0 [32, 64, 16]: r0[row, wHi, b] = in[b, wHi*32 + row]
    shared["r0"] = np.ascontiguousarray(
        ind[:, :, 0].reshape(B, 64, 32).transpose(2, 1, 0))

    # w0 [32, 128]: rows (2*wsub + k), cols (wsub*8 + co)
    w0 = np.zeros((32, 128), np.float32)
    for wsub in range(16):
        for k in range(2):
            w0[2 * wsub + k, wsub * 8:wsub * 8 + 8] = f0[k, 0, :]
    shared["w0"] = w0

    # packed levels 1..4 stacked: wpk [4, 128, 128]
    wpk = np.zeros((4, 128, 128), np.float32)
    for lvl in range(1, 5):
        cin = 2 ** (lvl - 1) * C
        cout = 2 ** lvl * C
        s_out = (128 // cin) // 2
        for wso in range(s_out):
            for k in range(2):
                wsi = 2 * wso + k
                wpk[lvl - 1, wsi * cin:(wsi + 1) * cin,
                    wso * cout:(wso + 1) * cout] = f[lvl][k]
    shared["wpk"] = wpk

    # w5/w6/w7 mega-packed [128, 10752] bf16 (kt-major per level), one tile
    w5v = f[5].astype(BF16).reshape(2, 1, 128, 256)
    w6v = f[6].astype(BF16).reshape(2, 2, 128, 512)
    w7v = f[7].astype(BF16).reshape(2, 4, 128, 1024)
    shared["wmid"] = np.ascontiguousarray(np.concatenate([
        w5v.transpose(2, 0, 1, 3).reshape(128, 512),
        w6v.transpose(2, 0, 1, 3).reshape(128, 2048),
        w7v.transpose(2, 0, 1, 3).reshape(128, 8192)], axis=1))

    # f8 is REPLICATED: co-major chunks [4, 128, kt=16, co=512], kt = k*8 + cit
    f8b = f[8].astype(BF16)
    w8full = np.stack([
        np.ascontiguousarray(
            f8b[:, :, c * 512:(c + 1) * 512]
            .reshape(2, 8, 128, 512).transpose(2, 0, 1, 3).reshape(128, 16, 512))
        for c in range(4)])
    shared["w8"] = w8full

    # f9 output-channel shards, packed into 4-ci-tile chunks:
    # [8, 128, 4, 512]; chunk m = k*4 + q, cit = q*4+j
    w9s = []
    f9b = f[9].astype(BF16)
    for r in range(NCORES):
        blk = f9b[:, :, r * 512:(r + 1) * 512]
        v = blk.reshape(2, 4, 4, 128, 512).transpose(0, 1, 3, 2, 4)
        w9s.append(np.ascontiguousarray(v.reshape(8, 128, 4, 512)))

    # f10 output-channel shards in float8_e3m4 with per-output-channel scales
    # (folded into fea_dense below): [16, 128, 4, 1024] fp8.
    # Chunk m = k*8 + j where j is the XOR exchange SLOT: the input-channel
    # block is q = r ^ j (slot j of the gathered x9 holds core (r^j)'s shard).
    s10 = np.max(np.abs(f[10]), axis=(0, 1)) / FP8_SCALE_TARGET  # [8192]
    f10q = (f[10] / s10[None, None, :]).astype(FP8)
    w10s = []
    for r in range(NCORES):
        v = f10q[:, :, r * 1024:(r + 1) * 1024].reshape(2, 8, 4, 128, 1024)
        chunks = []
        for m in range(16):
            k, j = divmod(m, 8)
            q = r ^ j
            chunks.append(v[k, q].transpose(1, 0, 2))     # [128, 4, 1024]
        w10s.append(np.ascontiguousarray(np.stack(chunks)))

    # fea_dense shard with the fp8 scales folded in, packed for the 4 PE
    # column groups: fdt[32*g + b, o, c] = fd_flat[o, g*256 + c] * s10[...]
    fds = []
    for r in range(NCORES):
        blk = fd[r * 128:(r + 1) * 128]                    # [128, 8, 2]
        flat = blk.transpose(2, 0, 1).reshape(2, 1024)     # [o, 1024]
        flat = flat * s10[r * 1024:(r + 1) * 1024][None, :]
        ft = np.zeros((128, 2, 256), np.float32)
        for g in range(4):
            ft[32 * g:32 * g + B] = np.broadcast_to(
                flat[None, :, 256 * g:256 * (g + 1)], (B, 2, 256))
        fds.append(np.ascontiguousarray(ft))

    in_maps = []
    for r in range(NCORES):
        m = dict(shared)
        m["w9"] = w9s[r]
        m["w10"] = w10s[r]
        m["fdt"] = fds[r]
        in_maps.append(m)
    return in_maps


# ---------------------------------------------------------------- bass build

def _build():
    import concourse.bass as bass
    import concourse.mybir as mybir
    import concourse.tile as tile
    from concourse import bacc

    f32 = mybir.dt.float32
    bf16 = mybir.dt.bfloat16
    fp8 = mybir.dt.float8e3
    RELU = mybir.ActivationFunctionType.Relu

    nc = bacc.Bacc("TRN2", target_bir_lowering=False, debug=False,
                   num_devices=NCORES)

    def inp(name, shape, dt=f32):
        return nc.dram_tensor(name, shape, dt, kind="ExternalInput").ap()

    r0 = inp("r0", [32, 64, 16])
    w0 = inp("w0", [32, 128])
    wpk = inp("wpk", [4, 128, 128])
    wmid = inp("wmid", [128, 10752], bf16)
    w8 = inp("w8", [4, 128, 16, 512], bf16)
    w9 = inp("w9", [8, 128, 4, 512], bf16)
    w10 = inp("w10", [16, 128, 4, 1024], fp8)
    fdt = inp("fdt", [128, 2, 256])
    out = nc.dram_tensor("out", [B, 128, 2], f32, kind="ExternalOutput").ap()

    xsems = [nc.alloc_semaphore(f"x9_xsem{d}") for d in range(3)]
    lsem = nc.alloc_semaphore("x9_lsem")
    psem = nc.alloc_semaphore("x9_psem")

    with tile.TileContext(nc) as tc:
        with (
            tc.tile_pool(name="const", bufs=1) as constp,
            tc.tile_pool(name="actp", bufs=3) as actp,
            tc.tile_pool(name="bigp", bufs=1) as bigp,
            tc.tile_pool(name="w7p", bufs=1) as w7p,
            tc.tile_pool(name="w8p", bufs=3) as w8p,
            tc.tile_pool(name="w9p", bufs=1) as w9p,
            tc.tile_pool(name="w10p", bufs=1) as w10p,
            tc.tile_pool(name="psA", bufs=2, space="PSUM") as psA,
            tc.tile_pool(name="psB", bufs=4, space="PSUM") as psB,
            tc.tile_pool(name="psC", bufs=1, space="PSUM") as psC,
            tc.tile_pool(name="dramp", bufs=1, space="DRAM") as dramp,
        ):
            # ---- resident loads, issued in consumption order
            r0sb = constp.tile([32, 64, 16], f32, name="r0sb")
            nc.sync.dma_start(r0sb[:], r0)
            w0sb = constp.tile([32, 128], f32, name="w0sb")
            nc.sync.dma_start(w0sb[:], w0)
            wpksb = constp.tile([128, 4, 128], f32, name="wpksb")
            nc.sync.dma_start(wpksb[:], wpk.rearrange("l p c -> p l c"))
            wmidsb = w7p.tile([128, 10752], bf16, name="wmidsb")
            # split so l5 can start before w6/w7 land
            nc.sync.dma_start(wmidsb[:, 0:512], wmid[:, 0:512])
            nc.sync.dma_start(wmidsb[:, 512:2560], wmid[:, 512:2560])
            nc.sync.dma_start(wmidsb[:, 2560:6656], wmid[:, 2560:6656])
            nc.sync.dma_start(wmidsb[:, 6656:10752], wmid[:, 6656:10752])
            w5sb = wmidsb[:, 0:512].rearrange("p (t c) -> p t c", c=256)
            w6sb = wmidsb[:, 512:2560].rearrange("p (t c) -> p t c", c=512)
            w7sb = wmidsb[:, 2560:10752].rearrange("p (t c) -> p t c", c=1024)

            # w9/w10 fully resident; slice DMAs let consumers start per-slice
            w9sb = w9p.tile([128, 8, 4, 512], bf16, name="w9sb")
            w10sb = w10p.tile([128, 16, 4, 1024], fp8, name="w10sb")

            # x9 exchange buffer (XOR slots): x9x[:, j] holds core (r^j)'s
            # [128, 4, 2, 16] shard; slot 0 is written locally by l9.
            x9x = bigp.tile([128, 8, 4, 2, 16], bf16, name="x9x")

            # Exchange = 3-round hypercube (XOR slots stay valid: in round d
            # I send my slots [0, 2^d) to peer r^2^d, landing in its slots
            # [2^d, 2^(d+1))). Each SWDGE ring entry costs 16 serially
            # processed lane descriptors (~6.3us): 3 entries beat 7.
            # Descriptor generation is slow (~6us + gpsimd ucode lib load):
            # run it EARLY in its own critical (criticals are all-engine
            # program-order barriers, hence the early placement; the rounds'
            # source-tensor reads happen at trigger time, sem-gated below).
            # Per-round remote sems: a fast far-partner must not satisfy an
            # earlier round's wait. no_gpsimd_drain skips a ~44us SWDGE
            # quiesce at critical exit.
            with tc.tile_critical(no_gpsimd_drain=True):
                nc.gpsimd.sem_clear(psem)
                for d in range(3):
                    e = 1 << d
                    rd = [None] * 8
                    # D2D engines deliver to tpb (requested ^ 2): pre-swap
                    v = e ^ 2 if e & 4 else e
                    rd[v] = (0, v)
                    nc.gpsimd.remote_dma_broadcast(
                        x9x[:, e:2 * e], x9x[:, 0:e],
                        remote_sem=xsems[d], local_sem=lsem, rdests=rd
                    ).then_inc(psem, 1)

            # ---- input conv + packed levels 1..4 (all [128, 64, 16])
            xprev = None
            for lvl in range(5):
                # x4 feeds the bf16 level-5 matmul, so cast at the relu
                xn = actp.tile([128, 64, 16], bf16 if lvl == 4 else f32,
                               name=f"x{lvl}", tag="xl")
                for ch in range(2):
                    ps = psA.tile([128, 32, 16], f32, name="psA", tag="psA")
                    if lvl == 0:
                        nc.tensor.matmul(
                            ps[:], w0sb[:], r0sb[:, ch * 32:(ch + 1) * 32, :],
                            start=True, stop=True)
                    else:
                        nc.tensor.matmul(
                            ps[:], wpksb[:, lvl - 1, :],
                            xprev[:, ch * 32:(ch + 1) * 32, :],
                            start=True, stop=True)
                    nc.scalar.activation(
                        xn[:, ch * 32:(ch + 1) * 32, :], ps[:], RELU)
                xprev = xn

            # ---- standard levels (orientation A, weights stationary)
            def std_level(xin, wsb, cin_t, cout_t, w_out, name, out_tile=None):
                # xin [128, cin_t, 2*w_out, 16]; wsb [128, 2*cin_t, co] with
                # kt = k*cin_t + cit; returns [128, cout_t, w_out, 16]
                if out_tile is None:
                    xn = actp.tile([128, cout_t, w_out, 16], bf16,
                                   name=name, tag="xl")
                else:
                    xn = out_tile
                for ct in range(cout_t):
                    ps = psA.tile([128, w_out, 16], f32, name="psA", tag="psA")
                    for cit in range(cin_t):
                        rhs2 = xin[:, cit].rearrange(
                            "p (w two) b -> p two w b", two=2)
                        for k in range(2):
                            nc.tensor.matmul(
                                ps[:],
                                wsb[:, k * cin_t + cit,
                                    ct * 128:(ct + 1) * 128],
                                rhs2[:, k],
                                start=(cit == 0 and k == 0),
                                stop=(cit == cin_t - 1 and k == 1))
                    nc.scalar.activation(xn[:, ct], ps[:], RELU)
                return xn

            x5 = std_level(xprev[:, None], w5sb, 1, 2, 32, "x5")
            x6 = std_level(x5, w6sb, 2, 4, 16, "x6")
            x7 = std_level(x6, w7sb, 4, 8, 8, "x7")

            # ---- level 8 REPLICATED (full 2048 cout), co-major weight stream
            x8sb = bigp.tile([128, 16, 4, 16], bf16, name="x8sb")
            w8cs = []
            for c in range(4):
                w8c = w8p.tile([128, 16, 512], bf16, name="w8c", tag="w8c")
                nc.sync.dma_start(w8c[:], w8[c])
                w8cs.append(w8c)
            # w9/w10/fdt descriptors queue behind w8 on the sync engine
            for m in range(8):
                nc.sync.dma_start(w9sb[:, m], w9[m])
            for m in range(16):
                nc.sync.dma_start(w10sb[:, m], w10[m])
            fdsb = constp.tile([128, 2, 256], f32, name="fdsb")
            nc.sync.dma_start(fdsb[:], fdt)

            for c in range(4):
                w8c = w8cs[c]
                for ctl in range(4):
                    ps = psA.tile([128, 4, 16], f32, name="psA", tag="psA")
                    for cit in range(8):
                        rhs2 = x7[:, cit].rearrange(
                            "p (w two) b -> p two w b", two=2)
                        for k in range(2):
                            nc.tensor.matmul(
                                ps[:],
                                w8c[:, k * 8 + cit, ctl * 128:(ctl + 1) * 128],
                                rhs2[:, k],
                                start=(cit == 0 and k == 0),
                                stop=(cit == 7 and k == 1))
                    nc.scalar.activation(x8sb[:, c * 4 + ctl], ps[:], RELU)

            # ---- level 9 (512-ch shard, resident weights, 4 accumulators)
            ps9 = [psB.tile([128, 2, 16], f32, name=f"ps9_{ct}", tag="psB")
                   for ct in range(4)]
            for m in range(8):
                k, q = divmod(m, 4)
                for j in range(4):
                    cit = q * 4 + j
                    rhs = x8sb[:, cit].rearrange(
                        "p (w two) b -> p two w b", two=2)[:, k]
                    for ct in range(4):
                        nc.tensor.matmul(
                            ps9[ct][:],
                            w9sb[:, m, j, ct * 128:(ct + 1) * 128],
                            rhs,
                            start=(m == 0 and j == 0),
                            stop=(m == 7 and j == 3))

            # ---- x9 local shard -> x9x slot 0, then fire the exchange.
            # No inter-core entry barrier needed: invocations are
            # host-serialized, xsem is cleared only post-consumption, and
            # early increments accumulate harmlessly.
            for ct in range(4):
                nc.scalar.activation(x9x[:, 0, ct], ps9[ct][:], RELU)

            # The token copy's read of x9x slot 0 gates the critical's entry
            # (the trigger instructions carry no tensor inputs, so without
            # it the sends would fire before l9's output exists). Rounds
            # trigger in ring-FIFO order; round d+1 waits for round d's
            # inbound data (its lanes read the slots that data fills).
            x9tok = bigp.tile([128, 4, 2, 16], bf16, name="x9tok")
            with tc.tile_critical(no_gpsimd_drain=True):
                nc.vector.tensor_scalar_add(x9tok[:], x9x[:, 0], 0.0)
                nc.gpsimd.wait_ge(psem, 3)
                nc.gpsimd.trigger_dma(count=1)
                nc.gpsimd.wait_ge(xsems[0], 2)
                nc.gpsimd.trigger_dma(count=1)
                nc.gpsimd.wait_ge(xsems[1], 2)
                nc.gpsimd.trigger_dma(count=1)

            # ---- level 10 (1024-ch shard, orientation B, fp8 weights moving,
            #      4-way PE column tiling: group g -> array cols 32g, PSUM
            #      partitions [32g, 32g+16), output cols [256g, 256(g+1))).
            #      Slot-0 (local) chunks run before the exchange completes.
            ps10 = psC.tile([128, 256], f32, name="ps10")

            def l10_chunk(m, xsrc):
                k, j = divmod(m, 8)
                for jj in range(4):
                    lhsT = xsrc[:, j, jj, k, :]
                    for g in range(4):
                        nc.tensor.matmul(
                            ps10[32 * g:32 * g + B, :], lhsT,
                            w10sb[:, m, jj, 256 * g:256 * (g + 1)],
                            start=(m == 0 and jj == 0),
                            stop=(m == 15 and jj == 3),
                            tile_position=(0, 32 * g),
                            skip_group_check=True)

            l10_chunk(0, x9x)
            l10_chunk(8, x9x)

            x9sb = bigp.tile([128, 8, 4, 2, 16], bf16, name="x9sb")
            with tc.tile_critical(no_gpsimd_drain=True):
                for d in range(3):
                    nc.vector.wait_ge(xsems[d], 2)
                    nc.vector.sem_clear(xsems[d])
                nc.vector.tensor_scalar_add(x9sb[:], x9x[:], 0.0)

            for m in range(16):
                if m not in (0, 8):
                    l10_chunk(m, x9sb)
            x10 = bigp.tile([128, 256], f32, name="x10")
            for g in range(4):
                nc.scalar.activation(
                    x10[32 * g:32 * g + B, :], ps10[32 * g:32 * g + B, :],
                    RELU)

            # ---- final per-block einsum on the vector engine
            osb = bigp.tile([128, 32, 2], f32, name="osb")
            for o in range(2):
                prod = bigp.tile([128, 256], f32, name=f"prod{o}")
                nc.vector.tensor_tensor(
                    prod[:], x10[:], fdsb[:, o, :], mybir.AluOpType.mult)
                nc.vector.tensor_reduce(
                    osb[:, :, o],
                    prod.rearrange("p (k c) -> p k c", c=8),
                    mybir.AxisListType.X, mybir.AluOpType.add)
            for g in range(4):
                nc.sync.dma_start(out[:, 32 * g:32 * (g + 1), :],
                                  osb[32 * g:32 * g + B, :, :])

            # 1-element AllGather, consumed by nobody: its presence makes
            # the runtime gang-dispatch the 8 cores (without any collective,
            # per-core launches stagger by milliseconds and the exchange
            # waits absorb the skew). Placed at the END, data-anchored on
            # osb so Tile cannot hoist it: collective_compute embeds a
            # completion wait that would otherwise block gpsimd mid-kernel.
            # By kernel end it has long completed (it runs concurrently on
            # the cc stream from each core's entry).
            agb_in = dramp.tile([1, 1], f32, name="agb_in")
            agb_out = dramp.tile([NCORES, 1], f32, name="agb_out",
                                 addr_space="Shared")
            nc.sync.dma_start(agb_in[:], osb[0:1, 0, 0:1])
            nc.gpsimd.collective_compute(
                "AllGather", mybir.AluOpType.bypass,
                replica_groups=[list(range(NCORES))],
                ins=[agb_in.opt()], outs=[agb_out.opt()])

    nc.compile()
    return nc


# ------------------------------------------------------------------- kernel

def kernel(**inputs):
    from concourse.bass_utils import run_bass_kernel_spmd

    in_maps = _host_prep(inputs)
    if "nc" not in _CACHE:
        _CACHE["nc"] = _build()
    nc = _CACHE["nc"]
    res = run_bass_kernel_spmd(nc, in_maps, core_ids=list(range(NCORES)))
    parts = [res.results[r]["out"] for r in range(NCORES)]  # each [16, 128, 2]
    full = np.concatenate(parts, axis=1)                    # [16, 1024, 2]
    return np.ascontiguousarray(full.reshape(B, 2048, 1).astype(np.float32))


# revision 22
# speedup vs baseline: 56.5860x; 1.1300x over previous
"""Trainium2 Bass kernel for the butterfly-CNN problem (nn_CNNLayer_30296699306356).

Network (see problem reference): input conv (k=2,s=2, 1->8 ch) + 10 butterfly
conv levels (k=2,s=2, channels double each level, relu, zero biases) + a
per-block dense matmul (1024 blocks of [8,2]) at the end.

Strategy (memory-regime; weights are ~358 MB fp32 dominated by levels 8-10):
  - Levels 5..9 run in bf16 (weights + activations, fp32 PSUM accumulation).
    Level 10 weights are float8e3 (e3m4) with per-output-channel scales that
    are folded into fea_dense on the host (relu commutes with positive
    scales), halving the dominant weight stream. Measured rel err ~1.4e-2
    (gate 2e-2, deterministic inputs).
  - Levels in..8 are replicated on all 8 cores; levels 9/10 shard the OUTPUT
    channels (1/8 of the dominant weight traffic per core). w9/w10 are fully
    RESIDENT in SBUF so the weight stream runs as one continuous FIFO.
  - x9 reassembly uses a hand-rolled one-shot all-to-all via
    remote_dma_broadcast (SBUF -> peer SBUF, ~5 us) instead of the gpsimd
    AllGather collective (~50 us of barrier+mesh latency). SPMD slot layout
    is XOR-based: slot j on core r holds core (r XOR j)'s x9 shard, which
    keeps every AP core-id-independent; the host permutes each core's w10
    chunk order to match. D2D engines deliver to tpb (requested ^ 2), so
    cross-die dests are requested pre-swapped (validated by probe.py).
  - Level 10 runs "orientation B" (x9 stationary, fp8 weights moving) with
    4-way PE column tiling: four independent 256-col matmul streams at array
    column offsets 0/32/64/96 accumulate into disjoint PSUM partition groups.
  - Final block einsum on the Vector engine across the 4 partition groups.

kernel(**inputs) takes the FULL unsharded inputs and returns the FULL output.
"""

import ml_dtypes
import numpy as np

NCORES = 8
B = 16
P = 128
C = 8
NLVL = 10
BF16 = ml_dtypes.bfloat16
FP8 = ml_dtypes.float8_e3m4
FP8_SCALE_TARGET = 14.0

_CACHE = {}


# ---------------------------------------------------------------- host prep

def _host_prep(inputs):
    """Build the per-core input maps (numpy only)."""
    ind = np.ascontiguousarray(np.asarray(inputs["in_data"], np.float32))
    f = {l: np.asarray(inputs[f"f{l}"], np.float32) for l in range(1, NLVL + 1)}
    f0 = np.asarray(inputs["in_filter"], np.float32)     # [2, 1, 8]
    fd = np.asarray(inputs["fea_dense"], np.float32)     # [1024, 8, 2]

    shared = {}
    # r0 [32, 64, 16]: r0[row, wHi, b] = in[b, wHi*32 + row]
    shared["r0"] = np.ascontiguousarray(
        ind[:, :, 0].reshape(B, 64, 32).transpose(2, 1, 0))

    # w0 [32, 128]: rows (2*wsub + k), cols (wsub*8 + co)
    w0 = np.zeros((32, 128), np.float32)
    for wsub in range(16):
        for k in range(2):
            w0[2 * wsub + k, wsub * 8:wsub * 8 + 8] = f0[k, 0, :]
    shared["w0"] = w0

    # packed levels 1..4 stacked: wpk [4, 128, 128]
    wpk = np.zeros((4, 128, 128), np.float32)
    for lvl in range(1, 5):
        cin = 2 ** (lvl - 1) * C
        cout = 2 ** lvl * C
        s_out = (128 // cin) // 2
        for wso in range(s_out):
            for k in range(2):
                wsi = 2 * wso + k
                wpk[lvl - 1, wsi * cin:(wsi + 1) * cin,
                    wso * cout:(wso + 1) * cout] = f[lvl][k]
    shared["wpk"] = wpk

    # w5/w6/w7 mega-packed [128, 10752] bf16 (kt-major per level), one tile
    w5v = f[5].astype(BF16).reshape(2, 1, 128, 256)
    w6v = f[6].astype(BF16).reshape(2, 2, 128, 512)
    w7v = f[7].astype(BF16).reshape(2, 4, 128, 1024)
    shared["wmid"] = np.ascontiguousarray(np.concatenate([
        w5v.transpose(2, 0, 1, 3).reshape(128, 512),
        w6v.transpose(2, 0, 1, 3).reshape(128, 2048),
        w7v.transpose(2, 0, 1, 3).reshape(128, 8192)], axis=1))

    # f8 is REPLICATED: co-major chunks [4, 128, kt=16, co=512], kt = k*8 + cit
    f8b = f[8].astype(BF16)
    w8full = np.stack([
        np.ascontiguousarray(
            f8b[:, :, c * 512:(c + 1) * 512]
            .reshape(2, 8, 128, 512).transpose(2, 0, 1, 3).reshape(128, 16, 512))
        for c in range(4)])
    shared["w8"] = w8full

    # f9 output-channel shards, packed into 4-ci-tile chunks:
    # [8, 128, 4, 512]; chunk m = k*4 + q, cit = q*4+j
    w9s = []
    f9b = f[9].astype(BF16)
    for r in range(NCORES):
        blk = f9b[:, :, r * 512:(r + 1) * 512]
        v = blk.reshape(2, 4, 4, 128, 512).transpose(0, 1, 3, 2, 4)
        w9s.append(np.ascontiguousarray(v.reshape(8, 128, 4, 512)))

    # f10 output-channel shards in float8_e3m4 with per-output-channel scales
    # (folded into fea_dense below): [16, 128, 4, 1024] fp8.
    # Chunk m = k*8 + j where j is the XOR exchange SLOT: the input-channel
    # block is q = r ^ j (slot j of the gathered x9 holds core (r^j)'s shard).
    s10 = np.max(np.abs(f[10]), axis=(0, 1)) / FP8_SCALE_TARGET  # [8192]
    f10q = (f[10] / s10[None, None, :]).astype(FP8)
    w10s = []
    for r in range(NCORES):
        v = f10q[:, :, r * 1024:(r + 1) * 1024].reshape(2, 8, 4, 128, 1024)
        chunks = []
        for m in range(16):
            k, j = divmod(m, 8)
            q = r ^ j
            chunks.append(v[k, q].transpose(1, 0, 2))     # [128, 4, 1024]
        w10s.append(np.ascontiguousarray(np.stack(chunks)))

    # fea_dense shard with the fp8 scales folded in, packed for the 4 PE
    # column groups: fdt[32*g + b, o, c] = fd_flat[o, g*256 + c] * s10[...]
    fds = []
    for r in range(NCORES):
        blk = fd[r * 128:(r + 1) * 128]                    # [128, 8, 2]
        flat = blk.transpose(2, 0, 1).reshape(2, 1024)     # [o, 1024]
        flat = flat * s10[r * 1024:(r + 1) * 1024][None, :]
        ft = np.zeros((128, 2, 256), np.float32)
        for g in range(4):
            ft[32 * g:32 * g + B] = np.broadcast_to(
                flat[None, :, 256 * g:256 * (g + 1)], (B, 2, 256))
        fds.append(np.ascontiguousarray(ft))

    in_maps = []
    for r in range(NCORES):
        m = dict(shared)
        m["w9"] = w9s[r]
        m["w10"] = w10s[r]
        m["fdt"] = fds[r]
        in_maps.append(m)
    return in_maps


# ---------------------------------------------------------------- bass build

def _build():
    import concourse.bass as bass
    import concourse.mybir as mybir
    import concourse.tile as tile
    from concourse import bacc

    f32 = mybir.dt.float32
    bf16 = mybir.dt.bfloat16
    fp8 = mybir.dt.float8e3
    RELU = mybir.ActivationFunctionType.Relu

    nc = bacc.Bacc("TRN2", target_bir_lowering=False, debug=False,
                   num_devices=NCORES)

    def inp(name, shape, dt=f32):
        return nc.dram_tensor(name, shape, dt, kind="ExternalInput").ap()

    r0 = inp("r0", [32, 64, 16])
    w0 = inp("w0", [32, 128])
    wpk = inp("wpk", [4, 128, 128])
    wmid = inp("wmid", [128, 10752], bf16)
    w8 = inp("w8", [4, 128, 16, 512], bf16)
    w9 = inp("w9", [8, 128, 4, 512], bf16)
    w10 = inp("w10", [16, 128, 4, 1024], fp8)
    fdt = inp("fdt", [128, 2, 256])
    out = nc.dram_tensor("out", [B, 128, 2], f32, kind="ExternalOutput").ap()

    xsems = [nc.alloc_semaphore(f"x9_xsem{d}") for d in range(3)]
    lsem = nc.alloc_semaphore("x9_lsem")
    psem = nc.alloc_semaphore("x9_psem")

    with tile.TileContext(nc) as tc:
        with (
            tc.tile_pool(name="const", bufs=1) as constp,
            tc.tile_pool(name="actp", bufs=3) as actp,
            tc.tile_pool(name="bigp", bufs=1) as bigp,
            tc.tile_pool(name="w7p", bufs=1) as w7p,
            tc.tile_pool(name="w8p", bufs=3) as w8p,
            tc.tile_pool(name="w9p", bufs=1) as w9p,
            tc.tile_pool(name="w10p", bufs=1) as w10p,
            tc.tile_pool(name="psA", bufs=2, space="PSUM") as psA,
            tc.tile_pool(name="psB", bufs=4, space="PSUM") as psB,
            tc.tile_pool(name="psC", bufs=1, space="PSUM") as psC,
            tc.tile_pool(name="dramp", bufs=1, space="DRAM") as dramp,
        ):
            # ---- resident loads, issued in consumption order
            r0sb = constp.tile([32, 64, 16], f32, name="r0sb")
            nc.sync.dma_start(r0sb[:], r0)

            # 1-element AllGather, consumed by nobody: its presence makes
            # the runtime gang-dispatch the 8 cores (without any collective,
            # per-core launches stagger by milliseconds). Anchored on r0sb
            # so it runs EARLY: collective_compute embeds a completion wait
            # on gpsimd, and the cc machinery takes ~60us from entry — by
            # anchoring early it completes before the exchange needs gpsimd,
            # instead of adding ~27us of pure tail after the output DMA.
            agb_in = dramp.tile([1, 1], f32, name="agb_in")
            agb_out = dramp.tile([NCORES, 1], f32, name="agb_out",
                                 addr_space="Shared")
            nc.sync.dma_start(agb_in[:], r0sb[0:1, 0, 0:1])
            nc.gpsimd.collective_compute(
                "AllGather", mybir.AluOpType.bypass,
                replica_groups=[list(range(NCORES))],
                ins=[agb_in.opt()], outs=[agb_out.opt()])
            w0sb = constp.tile([32, 128], f32, name="w0sb")
            nc.sync.dma_start(w0sb[:], w0)
            wpksb = constp.tile([128, 4, 128], f32, name="wpksb")
            nc.sync.dma_start(wpksb[:], wpk.rearrange("l p c -> p l c"))
            wmidsb = w7p.tile([128, 10752], bf16, name="wmidsb")
            # split so l5 can start before w6/w7 land
            nc.sync.dma_start(wmidsb[:, 0:512], wmid[:, 0:512])
            nc.sync.dma_start(wmidsb[:, 512:2560], wmid[:, 512:2560])
            nc.sync.dma_start(wmidsb[:, 2560:6656], wmid[:, 2560:6656])
            nc.sync.dma_start(wmidsb[:, 6656:10752], wmid[:, 6656:10752])
            w5sb = wmidsb[:, 0:512].rearrange("p (t c) -> p t c", c=256)
            w6sb = wmidsb[:, 512:2560].rearrange("p (t c) -> p t c", c=512)
            w7sb = wmidsb[:, 2560:10752].rearrange("p (t c) -> p t c", c=1024)

            # w9/w10 fully resident; slice DMAs let consumers start per-slice
            w9sb = w9p.tile([128, 8, 4, 512], bf16, name="w9sb")
            w10sb = w10p.tile([128, 16, 4, 1024], fp8, name="w10sb")

            # x9 exchange buffer (XOR slots): x9x[:, j] holds core (r^j)'s
            # [128, 4, 2, 16] shard; slot 0 is written locally by l9.
            x9x = bigp.tile([128, 8, 4, 2, 16], bf16, name="x9x")

            # Exchange = 3-round hypercube (XOR slots stay valid: in round d
            # I send my slots [0, 2^d) to peer r^2^d, landing in its slots
            # [2^d, 2^(d+1))). Each SWDGE ring entry costs 16 serially
            # processed lane descriptors (~6.3us): 3 entries beat 7.
            # Descriptor generation is slow (~6us + gpsimd ucode lib load):
            # run it EARLY in its own critical (criticals are all-engine
            # program-order barriers, hence the early placement; the rounds'
            # source-tensor reads happen at trigger time, sem-gated below).
            # Per-round remote sems: a fast far-partner must not satisfy an
            # earlier round's wait. no_gpsimd_drain skips a ~44us SWDGE
            # quiesce at critical exit.
            with tc.tile_critical(no_gpsimd_drain=True):
                nc.gpsimd.sem_clear(psem)
                for d in range(3):
                    e = 1 << d
                    rd = [None] * 8
                    # D2D engines deliver to tpb (requested ^ 2): pre-swap
                    v = e ^ 2 if e & 4 else e
                    rd[v] = (0, v)
                    nc.gpsimd.remote_dma_broadcast(
                        x9x[:, e:2 * e], x9x[:, 0:e],
                        remote_sem=xsems[d], local_sem=lsem, rdests=rd
                    ).then_inc(psem, 1)

            # ---- input conv + packed levels 1..4 (all [128, 64, 16])
            xprev = None
            for lvl in range(5):
                # x4 feeds the bf16 level-5 matmul, so cast at the relu
                xn = actp.tile([128, 64, 16], bf16 if lvl == 4 else f32,
                               name=f"x{lvl}", tag="xl")
                for ch in range(2):
                    ps = psA.tile([128, 32, 16], f32, name="psA", tag="psA")
                    if lvl == 0:
                        nc.tensor.matmul(
                            ps[:], w0sb[:], r0sb[:, ch * 32:(ch + 1) * 32, :],
                            start=True, stop=True)
                    else:
                        nc.tensor.matmul(
                            ps[:], wpksb[:, lvl - 1, :],
                            xprev[:, ch * 32:(ch + 1) * 32, :],
                            start=True, stop=True)
                    nc.scalar.activation(
                        xn[:, ch * 32:(ch + 1) * 32, :], ps[:], RELU)
                xprev = xn

            # ---- standard levels (orientation A, weights stationary)
            def std_level(xin, wsb, cin_t, cout_t, w_out, name, out_tile=None):
                # xin [128, cin_t, 2*w_out, 16]; wsb [128, 2*cin_t, co] with
                # kt = k*cin_t + cit; returns [128, cout_t, w_out, 16]
                if out_tile is None:
                    xn = actp.tile([128, cout_t, w_out, 16], bf16,
                                   name=name, tag="xl")
                else:
                    xn = out_tile
                for ct in range(cout_t):
                    ps = psA.tile([128, w_out, 16], f32, name="psA", tag="psA")
                    for cit in range(cin_t):
                        rhs2 = xin[:, cit].rearrange(
                            "p (w two) b -> p two w b", two=2)
                        for k in range(2):
                            nc.tensor.matmul(
                                ps[:],
                                wsb[:, k * cin_t + cit,
                                    ct * 128:(ct + 1) * 128],
                                rhs2[:, k],
                                start=(cit == 0 and k == 0),
                                stop=(cit == cin_t - 1 and k == 1))
                    nc.scalar.activation(xn[:, ct], ps[:], RELU)
                return xn

            x5 = std_level(xprev[:, None], w5sb, 1, 2, 32, "x5")
            x6 = std_level(x5, w6sb, 2, 4, 16, "x6")
            x7 = std_level(x6, w7sb, 4, 8, 8, "x7")

            # ---- level 8 REPLICATED (full 2048 cout), co-major weight stream
            x8sb = bigp.tile([128, 16, 4, 16], bf16, name="x8sb")
            w8cs = []
            for c in range(4):
                w8c = w8p.tile([128, 16, 512], bf16, name="w8c", tag="w8c")
                nc.sync.dma_start(w8c[:], w8[c])
                w8cs.append(w8c)
            # w9/w10/fdt descriptors issue from the SCALAR engine: a separate
            # HW queue (parallel HBM bandwidth), and it keeps the sync
            # engine's stream short so the exchange criticals (all-engine
            # entry barriers) are reachable as soon as x9 is ready.
            for m in range(8):
                nc.scalar.dma_start(w9sb[:, m], w9[m])
            for m in range(16):
                nc.scalar.dma_start(w10sb[:, m], w10[m])
            fdsb = constp.tile([128, 2, 256], f32, name="fdsb")
            nc.scalar.dma_start(fdsb[:], fdt)

            for c in range(4):
                w8c = w8cs[c]
                for ctl in range(4):
                    ps = psA.tile([128, 4, 16], f32, name="psA", tag="psA")
                    for cit in range(8):
                        rhs2 = x7[:, cit].rearrange(
                            "p (w two) b -> p two w b", two=2)
                        for k in range(2):
                            nc.tensor.matmul(
                                ps[:],
                                w8c[:, k * 8 + cit, ctl * 128:(ctl + 1) * 128],
                                rhs2[:, k],
                                start=(cit == 0 and k == 0),
                                stop=(cit == 7 and k == 1))
                    nc.scalar.activation(x8sb[:, c * 4 + ctl], ps[:], RELU)

            # ---- level 9 (512-ch shard, resident weights, 4 accumulators)
            ps9 = [psB.tile([128, 2, 16], f32, name=f"ps9_{ct}", tag="psB")
                   for ct in range(4)]
            for m in range(8):
                k, q = divmod(m, 4)
                for j in range(4):
                    cit = q * 4 + j
                    rhs = x8sb[:, cit].rearrange(
                        "p (w two) b -> p two w b", two=2)[:, k]
                    for ct in range(4):
                        nc.tensor.matmul(
                            ps9[ct][:],
                            w9sb[:, m, j, ct * 128:(ct + 1) * 128],
                            rhs,
                            start=(m == 0 and j == 0),
                            stop=(m == 7 and j == 3))

            # ---- x9 local shard -> x9x slot 0, then fire the exchange.
            # No inter-core entry barrier needed: invocations are
            # host-serialized, xsem is cleared only post-consumption, and
            # early increments accumulate harmlessly.
            for ct in range(4):
                nc.scalar.activation(x9x[:, 0, ct], ps9[ct][:], RELU)

            # The token copy's read of x9x slot 0 gates the critical's entry
            # (the trigger instructions carry no tensor inputs, so without
            # it the sends would fire before l9's output exists). Rounds
            # trigger in ring-FIFO order; round d+1 waits for round d's
            # inbound data (its lanes read the slots that data fills).
            x9tok = bigp.tile([128, 4, 2, 16], bf16, name="x9tok")
            with tc.tile_critical(no_gpsimd_drain=True):
                nc.vector.tensor_scalar_add(x9tok[:], x9x[:, 0], 0.0)
                nc.gpsimd.wait_ge(psem, 3)
                nc.gpsimd.trigger_dma(count=1)
                nc.gpsimd.wait_ge(xsems[0], 2)
                nc.gpsimd.trigger_dma(count=1)
                nc.gpsimd.wait_ge(xsems[1], 2)
                nc.gpsimd.trigger_dma(count=1)

            # ---- level 10 (1024-ch shard, orientation B, fp8 weights moving,
            #      4-way PE column tiling: group g -> array cols 32g, PSUM
            #      partitions [32g, 32g+16), output cols [256g, 256(g+1))).
            #      Slot-0 (local) chunks run before the exchange completes.
            ps10 = psC.tile([128, 256], f32, name="ps10")

            def l10_chunk(m, xsrc):
                k, j = divmod(m, 8)
                for jj in range(4):
                    lhsT = xsrc[:, j, jj, k, :]
                    for g in range(4):
                        nc.tensor.matmul(
                            ps10[32 * g:32 * g + B, :], lhsT,
                            w10sb[:, m, jj, 256 * g:256 * (g + 1)],
                            start=(m == 0 and jj == 0),
                            stop=(m == 15 and jj == 3),
                            tile_position=(0, 32 * g),
                            skip_group_check=True)

            l10_chunk(0, x9x)
            l10_chunk(8, x9x)

            x9sb = bigp.tile([128, 8, 4, 2, 16], bf16, name="x9sb")
            with tc.tile_critical(no_gpsimd_drain=True):
                for d in range(3):
                    nc.vector.wait_ge(xsems[d], 2)
                    nc.vector.sem_clear(xsems[d])
                nc.vector.tensor_scalar_add(x9sb[:], x9x[:], 0.0)

            for m in range(16):
                if m not in (0, 8):
                    l10_chunk(m, x9sb)
            x10 = bigp.tile([128, 256], f32, name="x10")
            for g in range(4):
                nc.scalar.activation(
                    x10[32 * g:32 * g + B, :], ps10[32 * g:32 * g + B, :],
                    RELU)

            # ---- final per-block einsum on the vector engine
            osb = bigp.tile([128, 32, 2], f32, name="osb")
            for o in range(2):
                prod = bigp.tile([128, 256], f32, name=f"prod{o}")
                nc.vector.tensor_tensor(
                    prod[:], x10[:], fdsb[:, o, :], mybir.AluOpType.mult)
                nc.vector.tensor_reduce(
                    osb[:, :, o],
                    prod.rearrange("p (k c) -> p k c", c=8),
                    mybir.AxisListType.X, mybir.AluOpType.add)
            for g in range(4):
                nc.sync.dma_start(out[:, 32 * g:32 * (g + 1), :],
                                  osb[32 * g:32 * g + B, :, :])

    nc.compile()
    return nc


# ------------------------------------------------------------------- kernel

def kernel(**inputs):
    from concourse.bass_utils import run_bass_kernel_spmd

    in_maps = _host_prep(inputs)
    if "nc" not in _CACHE:
        _CACHE["nc"] = _build()
    nc = _CACHE["nc"]
    res = run_bass_kernel_spmd(nc, in_maps, core_ids=list(range(NCORES)))
    parts = [res.results[r]["out"] for r in range(NCORES)]  # each [16, 128, 2]
    full = np.concatenate(parts, axis=1)                    # [16, 1024, 2]
    return np.ascontiguousarray(full.reshape(B, 2048, 1).astype(np.float32))
